# revision 16
# baseline (speedup 1.0000x reference)
"""Fused masked-softmax attention (DotProductAttention) for 8 TRN2 NeuronCores.

Problem: B=16 batches of Q[2048,64] @ K[2048,64]^T -> mask cols >= valid_len
to -1e6 -> softmax -> @ V[2048,64].

Work decomposition: each batch splits into 4 q-quarters of 512 rows (one
PSUM-bank-wide q-tile each) -> 64 independent units.  Units are sorted by
valid k-tile count nv = ceil(valid_len/128) and dealt into 8 SPMD slots of
8 units (one per core); the compiled program runs slot s with a static
nv_s = max over that slot's units.  Masking and the k-tiles a unit runs
past its own valid_len are handled by HOST-side zeroing of the V-side
operand (see below), so skipped tiles are exact and extra tiles are 0.

Per-unit kernel (all on-chip, scores never touch HBM):
  * Layout: S^T[k, q] so softmax's k-reduction becomes a matmul and the
    attn @ V contraction needs no transpose of the big matrix.
  * mm1 PAIRS: the contraction is only D=64 deep, so TWO k-tiles run
    CONCURRENTLY in disjoint 64-row strips of the 128x128 PE array
    (tile_position row groups): kTa packs even k-tiles in partitions 0-63
    and odd k-tiles in 64-127, qTa holds Q^T duplicated in both halves.
    Each pair costs ~one matmul of wall time instead of two.
  * masking: exp(score) for a masked/padding position is multiplied by a
    HOST-zeroed Vaug row in mm2, contributing exactly 0 to numerator and
    denominator.  No mask row, no extra instructions.
  * exp:  ACT engine, exp(0.125 * x) straight out of PSUM in merged
    N<=1536 activations, bf16 out.
  * mm2:  O^T_aug [65, 512q] = sum_k Vaug[ktile].T @ expS^T[ktile] with
    Vaug = [V | ones] (bf16, masked rows zeroed) -> row 64 accumulates the
    softmax denominator in fp32 PSUM.
  * The schedule is software-pipelined one activation-group ahead across
    slot boundaries: PE emits mm1(group i+1) before mm2(group i), so the
    ACT engine never waits at a slot transition.
  * finish: one DVE copy PSUM->SBUF (bf16), one DMA of O^T_aug [65,512]
    per unit.  Normalization (divide by the denominator row) and the
    final transpose to [512,64] happen on the HOST after the gather.
"""

import functools

import numpy as np
import ml_dtypes

import concourse.bacc as bacc
import concourse.tile as tile
from concourse import mybir
from concourse import bass_utils

B, LQ, LKV, D = 16, 2048, 2048, 64
N_CORES = 8
KT = 128            # k-tile (partition dim of S^T)
QT = 512            # q-rows per unit (= PSUM bank free dim)
NKT = LKV // KT     # 16
NPMAX = NKT // 2    # max mm1 pairs per unit
NSLOT = (B * LQ) // (N_CORES * QT)  # 8 units per core
GROUP = 3           # max k-tiles per PSUM tile / merged activation
WARM_MM = 4         # bf16 warm-up matmuls to engage the HAM clock ramp
F32 = mybir.dt.float32
BF16 = mybir.dt.bfloat16


def _widths(nv, first_single=False):
    """Split nv k-tiles into activation groups of width <=3, avoiding 1-wide
    groups (measured regression) where possible.  2-wide groups go FIRST so
    each slot's first activation has the shortest possible mm1 prefix.  The
    first slot processed leads with a 1-wide group: its k-tile 0 runs as an
    UNPAIRED matmul needing only the low half of qta and one 16KB kta chunk,
    so the first exp fires as early as the DMAs allow (ACT is idle at kernel
    start, making the extra instruction free)."""
    if first_single and nv > 1:
        return [1] + _widths(nv - 1)
    threes, rem = divmod(nv, 3)
    if rem == 0:
        return [3] * threes
    if rem == 2:
        return [2] + [3] * threes
    if threes >= 1:
        return [2, 2] + [3] * (threes - 1)
    return [nv]


def _mm1_ops(s, nv):
    """mm1 schedule for a slot: list of (pair_col, [(half, tile_n), ...]).
    Normal slots pack even tiles in array rows 0-63 and odd tiles in rows
    64-127.  Slot 0 is SHIFTED: tile 0 runs alone first (cheap head), then
    pairs (1,2), (3,4), ... — matching the host-side kta layout."""
    if s == 0:
        ops = [(0, [(0, 0)])]
        for p in range(1, nv // 2 + 1):
            items = [(0, 2 * p - 1)]
            if 2 * p < nv:
                items.append((1, 2 * p))
            ops.append((p, items))
        return ops
    ops = []
    for p in range((nv + 1) // 2):
        items = [(0, 2 * p)]
        if 2 * p + 1 < nv:
            items.append((1, 2 * p + 1))
        ops.append((p, items))
    return ops


@functools.lru_cache(maxsize=4)
def _build_module(nv_slots):
    nc = bacc.Bacc(None)
    qta_d = nc.dram_tensor("qta", [NSLOT, 128, QT], BF16, kind="ExternalInput")
    kta_d = nc.dram_tensor(
        "kta", [NSLOT, 128, (NPMAX + 1) * KT], BF16, kind="ExternalInput"
    )
    vau_d = nc.dram_tensor("vaug", [128, NSLOT * NKT * (D + 1)], BF16, kind="ExternalInput")
    out_d = nc.dram_tensor("o", [NSLOT, D + 1, QT], BF16, kind="ExternalOutput")

    np_slots = [
        (nv // 2 + 1) if s == 0 else (nv + 1) // 2 for s, nv in enumerate(nv_slots)
    ]

    slot_groups = []
    for si, nv in enumerate(nv_slots):
        groups, g = [], 0
        for w in _widths(nv, first_single=(si == 0)):
            groups.append((g, w))
            g += w
        assert g == nv
        slot_groups.append(groups)

    with tile.TileContext(nc) as tc:
        with (
            tc.tile_pool(name="weights", bufs=1) as wpool,
            tc.tile_pool(name="exps", bufs=3) as epool,
            tc.tile_pool(name="outs", bufs=3) as opool,
            tc.tile_pool(name="ps_s", bufs=2, space="PSUM") as ps_s,
            tc.tile_pool(name="ps_o", bufs=2, space="PSUM") as ps_o,
        ):
            # Slots are sorted big -> small by _plan: start with the largest
            # (small prefetched head chunk), end with the smallest (minimal
            # exposed tail chain: last exp -> mm2s -> copy -> DMA).
            proc_order = list(range(NSLOT))

            kta_s = [
                wpool.tile([128, np_slots[s] * KT], BF16, tag=f"kta{s}", name=f"kta{s}")
                for s in range(NSLOT)
            ]
            qta_s = [
                wpool.tile([128, QT], BF16, tag=f"qta{s}", name=f"qta{s}")
                for s in range(NSLOT)
            ]
            vaug_s = [
                wpool.tile(
                    [128, nv_slots[s] * (D + 1)], BF16, tag=f"vaug{s}", name=f"vaug{s}"
                )
                for s in range(NSLOT)
            ]

            # PE warm-up: dense bf16 matmuls on a memset dummy tile while the
            # first input DMA is in flight (HAM clock ramps 1.2 -> 2.4 GHz
            # only after sustained PE activity).  GPSIMD finishes its preamble
            # first, so it does the memset; the count is capped so the last
            # warm matmul retires before the first input data can land.
            dummy = wpool.tile([128, QT], BF16, tag="dummy", name="dummy")
            nc.gpsimd.memset(dummy, 0)
            warm = ps_s.tile([128, GROUP * QT], F32, tag="st", name="warm")
            for _ in range(WARM_MM):
                nc.tensor.matmul(
                    warm[:, :QT], lhsT=dummy[:, :128], rhs=dummy, start=True, stop=True
                )

            # The first matmul (slot 0's unpaired k-tile 0) needs only the low
            # half of qta plus one 16KB kta chunk — they go out in parallel on
            # the two HWDGE rings ahead of everything else.  The scalar ring
            # only carries DMAs that finish issuing before the first ACTIVATE
            # enters its (strict FIFO) queue.
            nc.sync.dma_start(out=qta_s[0][0:64, :], in_=qta_d[0, 0:64, :])
            nc.scalar.dma_start(out=kta_s[0][0:64, :KT], in_=kta_d[0, 0:64, :KT])
            nc.sync.dma_start(out=qta_s[0][64:128, :], in_=qta_d[0, 64:128, :])
            nc.scalar.dma_start(
                out=vaug_s[0], in_=vau_d[:, : nv_slots[0] * (D + 1)]
            )
            nc.sync.dma_start(
                out=kta_s[0][:, KT:], in_=kta_d[0, :, KT : np_slots[0] * KT]
            )
            nc.scalar.dma_start(out=qta_s[1], in_=qta_d[1])
            for s in proc_order[1:]:
                if s != 1:
                    nc.sync.dma_start(out=qta_s[s], in_=qta_d[s])
                nc.sync.dma_start(out=kta_s[s], in_=kta_d[s, :, : np_slots[s] * KT])
                nc.sync.dma_start(
                    out=vaug_s[s],
                    in_=vau_d[:, s * NKT * (D + 1) : (s * NKT + nv_slots[s]) * (D + 1)],
                )

            exps_t, po_t, st_t = {}, {}, {}
            emitted = []  # (s, g0, w) in exp emission order

            def mm2_group(s, g, w):
                nv = nv_slots[s]
                for j in range(w):
                    n = g + j
                    nc.tensor.matmul(
                        po_t[s],
                        lhsT=vaug_s[s][:, n * (D + 1) : (n + 1) * (D + 1)],
                        rhs=exps_t[s][:, n * QT : (n + 1) * QT],
                        start=(n == 0),
                        stop=(n == nv - 1),
                        skip_group_check=True,
                    )
                if g + w == nv:
                    # bf16 staging halves the output DMA; the host divides
                    # numerator by denominator in fp32 after the cast.
                    ob = opool.tile([D + 1, QT], BF16, tag="ob", name=f"ob{s}")
                    nc.vector.tensor_copy(ob, po_t[s])
                    nc.sync.dma_start(out=out_d[s], in_=ob)

            def emit_exp(s, gi):
                g, w = slot_groups[s][gi]
                nc.scalar.activation(
                    out=exps_t[s][:, g * QT : (g + w) * QT],
                    in_=st_t[(s, gi)][:, : w * QT],
                    func=mybir.ActivationFunctionType.Exp,
                    scale=0.125,
                )
                emitted.append((s, g, w))
                if len(emitted) >= 2:
                    mm2_group(*emitted[-2])

            for s in proc_order:
                nv = nv_slots[s]
                exps_t[s] = epool.tile([128, nv * QT], BF16, tag="exps", name=f"exps{s}")
                po_t[s] = ps_o.tile([D + 1, QT], F32, tag="po", name=f"po{s}")

                # tile index -> (group idx, slot-within-group)
                t2g = {}
                for gi, (g, w) in enumerate(slot_groups[s]):
                    for j in range(w):
                        t2g[g + j] = (gi, j)

                def st_slice(n):
                    gi, j = t2g[n]
                    if (s, gi) not in st_t:
                        st_t[(s, gi)] = ps_s.tile(
                            [128, GROUP * QT], F32, tag="st", name=f"st{s}_{gi}"
                        )
                    return st_t[(s, gi)][:, j * QT : (j + 1) * QT]

                # mm1 ops: pairs of k-tiles run concurrently in the two
                # 64-row strips of the PE array (slot 0 leads with a single).
                for p, items in _mm1_ops(s, nv):
                    for half, n in items:
                        nc.tensor.matmul(
                            st_slice(n),
                            lhsT=kta_s[s][64 * half : 64 * half + 64, p * KT : (p + 1) * KT],
                            rhs=qta_s[s][64 * half : 64 * half + 64, :],
                            start=True,
                            stop=True,
                        )
                    for _, n in items:
                        gi, j = t2g[n]
                        if j == slot_groups[s][gi][1] - 1:
                            emit_exp(s, gi)
            mm2_group(*emitted[-1])

    nc.compile()
    return nc


def _plan(valid_lens):
    """Sort the 64 (batch, q-quarter) units by valid k-tile count and deal
    them into NSLOT slots of one unit per core.  Returns (core_units,
    nv_slots) where core_units[c][s] = (batch, quarter)."""
    VL = np.asarray(valid_lens).astype(np.int64)
    nv = np.maximum(1, np.minimum(NKT, (VL + KT - 1) // KT))
    qpb = LQ // QT  # quarters per batch
    unit_nv = np.repeat(nv, qpb)
    order = np.argsort(-unit_nv, kind="stable")
    core_units = [
        [(int(order[NSLOT * s + c]) // qpb, int(order[NSLOT * s + c]) % qpb) for s in range(NSLOT)]
        for c in range(N_CORES)
    ]
    nv_slots = tuple(int(unit_nv[order[NSLOT * s]]) for s in range(NSLOT))
    return core_units, nv_slots


def _shard_inputs(queries, keys, values, valid_lens, core_units):
    """Host-side layout per core: stacked per-unit augmented operands."""
    Q = np.asarray(queries, dtype=np.float32)
    K = np.asarray(keys, dtype=np.float32)
    V = np.asarray(values, dtype=np.float32)
    VL = np.asarray(valid_lens).astype(np.int64)

    in_maps = []
    for c in range(N_CORES):
        qta = np.empty((NSLOT, 128, QT), np.float32)
        kta = np.zeros((NSLOT, 128, (NPMAX + 1) * KT), np.float32)
        va = np.empty((128, NSLOT * NKT * (D + 1)), np.float32)
        for s, (b, qt) in enumerate(core_units[c]):
            qT = Q[b, qt * QT : (qt + 1) * QT, :].T          # [64, 512]
            qta[s, 0:64] = qT
            qta[s, 64:128] = qT
            tiles = K[b].T.reshape(D, NKT, KT)               # [64, 16, 128]
            if s == 0:
                # shifted layout: tile 0 alone in pair-col 0 (low half),
                # then pairs (1,2), (3,4), ... in cols 1..8
                lo = [0] + list(range(1, NKT, 2))            # 0,1,3,..,15
                hi = list(range(2, NKT, 2))                  # 2,4,..,14
                kta[s, 0:64, : len(lo) * KT] = (
                    tiles[:, lo, :].reshape(D, -1)
                )
                kta[s, 64:128, KT : (1 + len(hi)) * KT] = (
                    tiles[:, hi, :].reshape(D, -1)
                )
            else:
                kta[s, 0:64, : NPMAX * KT] = tiles[:, 0::2, :].reshape(D, -1)
                kta[s, 64:128, : NPMAX * KT] = tiles[:, 1::2, :].reshape(D, -1)
            # Vaug = [V | ones] with every row at or past valid_len zeroed:
            # masked/padding scores then contribute exactly 0 to both the
            # numerator and the denominator in mm2.
            vb = np.concatenate([V[b], np.ones((LKV, 1), np.float32)], axis=-1)
            vb[VL[b] :, :] = 0.0
            va[:, s * NKT * (D + 1) : (s + 1) * NKT * (D + 1)] = (
                vb.reshape(NKT, KT, D + 1).transpose(1, 0, 2).reshape(128, -1)
            )
        in_maps.append(
            {
                "qta": qta.astype(ml_dtypes.bfloat16),
                "kta": kta.astype(ml_dtypes.bfloat16),
                "vaug": va.astype(ml_dtypes.bfloat16),
            }
        )
    return in_maps


def _unpack(results, core_units):
    """Host-side finish: normalize by the accumulated denominator row and
    transpose each unit's O^T_aug [65, 512] back to [512, 64]."""
    out = np.empty((B, LQ, D), np.float32)
    for c in range(N_CORES):
        o = np.asarray(results[c]["o"], dtype=np.float32).reshape(NSLOT, D + 1, QT)
        normed = o[:, :D, :] / o[:, D : D + 1, :]          # [NSLOT, 64, 512]
        normed = normed.transpose(0, 2, 1)                 # [NSLOT, 512, 64]
        for s, (b, qt) in enumerate(core_units[c]):
            out[b, qt * QT : (qt + 1) * QT, :] = normed[s]
    return out


def kernel(queries, keys, values, valid_lens):
    core_units, nv_slots = _plan(valid_lens)
    nc = _build_module(nv_slots)
    in_maps = _shard_inputs(queries, keys, values, valid_lens, core_units)
    res = bass_utils.run_bass_kernel_spmd(nc, in_maps, core_ids=list(range(N_CORES)))
    return _unpack(res.results, core_units)


# revision 18
# speedup vs baseline: 1.0116x; 1.0116x over previous
"""Fused masked-softmax attention (DotProductAttention) for 8 TRN2 NeuronCores.

Problem: B=16 batches of Q[2048,64] @ K[2048,64]^T -> mask cols >= valid_len
to -1e6 -> softmax -> @ V[2048,64].

Work decomposition: each batch splits into 4 q-quarters of 512 rows (one
PSUM-bank-wide q-tile each) -> 64 independent units.  Units are sorted by
valid k-tile count nv = ceil(valid_len/128) and dealt into 8 SPMD slots of
8 units (one per core); the compiled program runs slot s with a static
nv_s = max over that slot's units.  Masking and the k-tiles a unit runs
past its own valid_len are handled by HOST-side zeroing of the V-side
operand (see below), so skipped tiles are exact and extra tiles are 0.

Per-unit kernel (all on-chip, scores never touch HBM):
  * Layout: S^T[k, q] so softmax's k-reduction becomes a matmul and the
    attn @ V contraction needs no transpose of the big matrix.
  * mm1 PAIRS: the contraction is only D=64 deep, so TWO k-tiles run
    CONCURRENTLY in disjoint 64-row strips of the 128x128 PE array
    (tile_position row groups): kTa packs even k-tiles in partitions 0-63
    and odd k-tiles in 64-127, qTa holds Q^T duplicated in both halves.
    Each pair costs ~one matmul of wall time instead of two.
  * masking: exp(score) for a masked/padding position is multiplied by a
    HOST-zeroed Vaug row in mm2, contributing exactly 0 to numerator and
    denominator.  No mask row, no extra instructions.
  * exp:  ACT engine, exp(0.125 * x) straight out of PSUM in merged
    N<=1536 activations, bf16 out.
  * mm2:  O^T_aug [65, 512q] = sum_k Vaug[ktile].T @ expS^T[ktile] with
    Vaug = [V | ones] (bf16, masked rows zeroed) -> row 64 accumulates the
    softmax denominator in fp32 PSUM.
  * The schedule is software-pipelined one activation-group ahead across
    slot boundaries: PE emits mm1(group i+1) before mm2(group i), so the
    ACT engine never waits at a slot transition.
  * finish: one DVE copy PSUM->SBUF (bf16), one DMA of O^T_aug [65,512]
    per unit.  Normalization (divide by the denominator row) and the
    final transpose to [512,64] happen on the HOST after the gather.
"""

import functools

import numpy as np
import ml_dtypes

import concourse.bacc as bacc
import concourse.tile as tile
from concourse import mybir
from concourse import bass_utils

B, LQ, LKV, D = 16, 2048, 2048, 64
N_CORES = 8
KT = 128            # k-tile (partition dim of S^T)
QT = 512            # q-rows per unit (= PSUM bank free dim)
NKT = LKV // KT     # 16
NPMAX = NKT // 2    # max mm1 pairs per unit
NSLOT = (B * LQ) // (N_CORES * QT)  # 8 units per core
GROUP = 3           # max k-tiles per PSUM tile / merged activation
# bf16 warm-up matmul free-dims: long enough continuous PE activity to engage
# the HAM clock ramp (4096-cycle busy window), tapering to short matmuls near
# the expected first-data arrival so the first real matmul isn't held up.
WARM_NS = [512] * 5 + [128] * 4
F32 = mybir.dt.float32
BF16 = mybir.dt.bfloat16


def _widths(nv, first_single=False):
    """Split nv k-tiles into activation groups of width <=3, avoiding 1-wide
    groups (measured regression) where possible.  2-wide groups go FIRST so
    each slot's first activation has the shortest possible mm1 prefix.  The
    first slot processed leads with a 1-wide group: its k-tile 0 runs as an
    UNPAIRED matmul needing only the low half of qta and one 16KB kta chunk,
    so the first exp fires as early as the DMAs allow (ACT is idle at kernel
    start, making the extra instruction free)."""
    if first_single and nv > 1:
        return [1] + _widths(nv - 1)
    threes, rem = divmod(nv, 3)
    if rem == 0:
        return [3] * threes
    if rem == 2:
        return [2] + [3] * threes
    if threes >= 1:
        return [2, 2] + [3] * (threes - 1)
    return [nv]


def _mm1_ops(s, nv):
    """mm1 schedule for a slot: list of (pair_col, [(half, tile_n), ...]).
    Normal slots pack even tiles in array rows 0-63 and odd tiles in rows
    64-127.  Slot 0 is SHIFTED: tile 0 runs alone first (cheap head), then
    pairs (1,2), (3,4), ... — matching the host-side kta layout."""
    if s == 0:
        ops = [(0, [(0, 0)])]
        for p in range(1, nv // 2 + 1):
            items = [(0, 2 * p - 1)]
            if 2 * p < nv:
                items.append((1, 2 * p))
            ops.append((p, items))
        return ops
    ops = []
    for p in range((nv + 1) // 2):
        items = [(0, 2 * p)]
        if 2 * p + 1 < nv:
            items.append((1, 2 * p + 1))
        ops.append((p, items))
    return ops


@functools.lru_cache(maxsize=4)
def _build_module(nv_slots):
    nc = bacc.Bacc(None)
    qta_d = nc.dram_tensor("qta", [NSLOT, 128, QT], BF16, kind="ExternalInput")
    kta_d = nc.dram_tensor(
        "kta", [NSLOT, 128, (NPMAX + 1) * KT], BF16, kind="ExternalInput"
    )
    vau_d = nc.dram_tensor("vaug", [128, NSLOT * NKT * (D + 1)], BF16, kind="ExternalInput")
    out_d = nc.dram_tensor("o", [NSLOT, D + 1, QT], BF16, kind="ExternalOutput")

    np_slots = [
        (nv // 2 + 1) if s == 0 else (nv + 1) // 2 for s, nv in enumerate(nv_slots)
    ]

    slot_groups = []
    for si, nv in enumerate(nv_slots):
        groups, g = [], 0
        for w in _widths(nv, first_single=(si == 0)):
            groups.append((g, w))
            g += w
        assert g == nv
        slot_groups.append(groups)

    with tile.TileContext(nc) as tc:
        with (
            tc.tile_pool(name="weights", bufs=1) as wpool,
            tc.tile_pool(name="exps", bufs=3) as epool,
            tc.tile_pool(name="outs", bufs=3) as opool,
            tc.tile_pool(name="ps_s", bufs=2, space="PSUM") as ps_s,
            tc.tile_pool(name="ps_o", bufs=2, space="PSUM") as ps_o,
        ):
            # Slots are sorted big -> small by _plan: start with the largest
            # (small prefetched head chunk), end with the smallest (minimal
            # exposed tail chain: last exp -> mm2s -> copy -> DMA).
            proc_order = list(range(NSLOT))

            kta_s = [
                wpool.tile([128, np_slots[s] * KT], BF16, tag=f"kta{s}", name=f"kta{s}")
                for s in range(NSLOT)
            ]
            qta_s = [
                wpool.tile([128, QT], BF16, tag=f"qta{s}", name=f"qta{s}")
                for s in range(NSLOT)
            ]
            vaug_s = [
                wpool.tile(
                    [128, nv_slots[s] * (D + 1)], BF16, tag=f"vaug{s}", name=f"vaug{s}"
                )
                for s in range(NSLOT)
            ]

            # PE warm-up: dense bf16 matmuls on a memset dummy tile while the
            # first input DMA is in flight (HAM clock ramps 1.2 -> 2.4 GHz
            # only after sustained PE activity).  GPSIMD finishes its preamble
            # first, so it does the memset; the count is capped so the last
            # warm matmul retires before the first input data can land.
            dummy = wpool.tile([128, QT], BF16, tag="dummy", name="dummy")
            nc.gpsimd.memset(dummy, 0)
            warm = ps_s.tile([128, GROUP * QT], F32, tag="st", name="warm")
            for n in WARM_NS:
                nc.tensor.matmul(
                    warm[:, :n], lhsT=dummy[:, :128], rhs=dummy[:, :n], start=True, stop=True
                )

            # The first matmul (slot 0's unpaired k-tile 0) needs only the low
            # half of qta plus one 16KB kta chunk — they go out in parallel on
            # the two HWDGE rings ahead of everything else.  The scalar ring
            # only carries DMAs that finish issuing before the first ACTIVATE
            # enters its (strict FIFO) queue.
            nc.sync.dma_start(out=qta_s[0][0:64, :], in_=qta_d[0, 0:64, :])
            nc.scalar.dma_start(out=kta_s[0][0:64, :KT], in_=kta_d[0, 0:64, :KT])
            nc.sync.dma_start(out=qta_s[0][64:128, :], in_=qta_d[0, 64:128, :])
            nc.scalar.dma_start(
                out=vaug_s[0], in_=vau_d[:, : nv_slots[0] * (D + 1)]
            )
            nc.sync.dma_start(
                out=kta_s[0][:, KT:], in_=kta_d[0, :, KT : np_slots[0] * KT]
            )
            nc.scalar.dma_start(out=qta_s[1], in_=qta_d[1])
            for s in proc_order[1:]:
                if s != 1:
                    nc.sync.dma_start(out=qta_s[s], in_=qta_d[s])
                nc.sync.dma_start(out=kta_s[s], in_=kta_d[s, :, : np_slots[s] * KT])
                nc.sync.dma_start(
                    out=vaug_s[s],
                    in_=vau_d[:, s * NKT * (D + 1) : (s * NKT + nv_slots[s]) * (D + 1)],
                )

            exps_t, po_t, st_t = {}, {}, {}
            emitted = []  # (s, g0, w) in exp emission order

            def mm2_group(s, g, w):
                nv = nv_slots[s]
                for j in range(w):
                    n = g + j
                    nc.tensor.matmul(
                        po_t[s],
                        lhsT=vaug_s[s][:, n * (D + 1) : (n + 1) * (D + 1)],
                        rhs=exps_t[s][:, n * QT : (n + 1) * QT],
                        start=(n == 0),
                        stop=(n == nv - 1),
                        skip_group_check=True,
                    )
                if g + w == nv:
                    # bf16 staging halves the output DMA; the host divides
                    # numerator by denominator in fp32 after the cast.
                    ob = opool.tile([D + 1, QT], BF16, tag="ob", name=f"ob{s}")
                    nc.vector.tensor_copy(ob, po_t[s])
                    nc.sync.dma_start(out=out_d[s], in_=ob)

            def emit_exp(s, gi):
                g, w = slot_groups[s][gi]
                nc.scalar.activation(
                    out=exps_t[s][:, g * QT : (g + w) * QT],
                    in_=st_t[(s, gi)][:, : w * QT],
                    func=mybir.ActivationFunctionType.Exp,
                    scale=0.125,
                )
                emitted.append((s, g, w))
                if len(emitted) >= 2:
                    mm2_group(*emitted[-2])

            for s in proc_order:
                nv = nv_slots[s]
                exps_t[s] = epool.tile([128, nv * QT], BF16, tag="exps", name=f"exps{s}")
                po_t[s] = ps_o.tile([D + 1, QT], F32, tag="po", name=f"po{s}")

                # tile index -> (group idx, slot-within-group)
                t2g = {}
                for gi, (g, w) in enumerate(slot_groups[s]):
                    for j in range(w):
                        t2g[g + j] = (gi, j)

                def st_slice(n):
                    gi, j = t2g[n]
                    if (s, gi) not in st_t:
                        st_t[(s, gi)] = ps_s.tile(
                            [128, GROUP * QT], F32, tag="st", name=f"st{s}_{gi}"
                        )
                    return st_t[(s, gi)][:, j * QT : (j + 1) * QT]

                # mm1 ops: pairs of k-tiles run concurrently in the two
                # 64-row strips of the PE array (slot 0 leads with a single).
                for p, items in _mm1_ops(s, nv):
                    for half, n in items:
                        nc.tensor.matmul(
                            st_slice(n),
                            lhsT=kta_s[s][64 * half : 64 * half + 64, p * KT : (p + 1) * KT],
                            rhs=qta_s[s][64 * half : 64 * half + 64, :],
                            start=True,
                            stop=True,
                        )
                    for _, n in items:
                        gi, j = t2g[n]
                        if j == slot_groups[s][gi][1] - 1:
                            emit_exp(s, gi)
            mm2_group(*emitted[-1])

    nc.compile()
    return nc


def _plan(valid_lens):
    """Sort the 64 (batch, q-quarter) units by valid k-tile count and deal
    them into NSLOT slots of one unit per core.  Returns (core_units,
    nv_slots) where core_units[c][s] = (batch, quarter)."""
    VL = np.asarray(valid_lens).astype(np.int64)
    nv = np.maximum(1, np.minimum(NKT, (VL + KT - 1) // KT))
    qpb = LQ // QT  # quarters per batch
    unit_nv = np.repeat(nv, qpb)
    order = np.argsort(-unit_nv, kind="stable")
    core_units = [
        [(int(order[NSLOT * s + c]) // qpb, int(order[NSLOT * s + c]) % qpb) for s in range(NSLOT)]
        for c in range(N_CORES)
    ]
    nv_slots = tuple(int(unit_nv[order[NSLOT * s]]) for s in range(NSLOT))
    return core_units, nv_slots


def _shard_inputs(queries, keys, values, valid_lens, core_units):
    """Host-side layout per core: stacked per-unit augmented operands."""
    Q = np.asarray(queries, dtype=np.float32)
    K = np.asarray(keys, dtype=np.float32)
    V = np.asarray(values, dtype=np.float32)
    VL = np.asarray(valid_lens).astype(np.int64)

    in_maps = []
    for c in range(N_CORES):
        qta = np.empty((NSLOT, 128, QT), np.float32)
        kta = np.zeros((NSLOT, 128, (NPMAX + 1) * KT), np.float32)
        va = np.empty((128, NSLOT * NKT * (D + 1)), np.float32)
        for s, (b, qt) in enumerate(core_units[c]):
            qT = Q[b, qt * QT : (qt + 1) * QT, :].T          # [64, 512]
            qta[s, 0:64] = qT
            qta[s, 64:128] = qT
            tiles = K[b].T.reshape(D, NKT, KT)               # [64, 16, 128]
            if s == 0:
                # shifted layout: tile 0 alone in pair-col 0 (low half),
                # then pairs (1,2), (3,4), ... in cols 1..8
                lo = [0] + list(range(1, NKT, 2))            # 0,1,3,..,15
                hi = list(range(2, NKT, 2))                  # 2,4,..,14
                kta[s, 0:64, : len(lo) * KT] = (
                    tiles[:, lo, :].reshape(D, -1)
                )
                kta[s, 64:128, KT : (1 + len(hi)) * KT] = (
                    tiles[:, hi, :].reshape(D, -1)
                )
            else:
                kta[s, 0:64, : NPMAX * KT] = tiles[:, 0::2, :].reshape(D, -1)
                kta[s, 64:128, : NPMAX * KT] = tiles[:, 1::2, :].reshape(D, -1)
            # Vaug = [V | ones] with every row at or past valid_len zeroed:
            # masked/padding scores then contribute exactly 0 to both the
            # numerator and the denominator in mm2.
            vb = np.concatenate([V[b], np.ones((LKV, 1), np.float32)], axis=-1)
            vb[VL[b] :, :] = 0.0
            va[:, s * NKT * (D + 1) : (s + 1) * NKT * (D + 1)] = (
                vb.reshape(NKT, KT, D + 1).transpose(1, 0, 2).reshape(128, -1)
            )
        in_maps.append(
            {
                "qta": qta.astype(ml_dtypes.bfloat16),
                "kta": kta.astype(ml_dtypes.bfloat16),
                "vaug": va.astype(ml_dtypes.bfloat16),
            }
        )
    return in_maps


def _unpack(results, core_units):
    """Host-side finish: normalize by the accumulated denominator row and
    transpose each unit's O^T_aug [65, 512] back to [512, 64]."""
    out = np.empty((B, LQ, D), np.float32)
    for c in range(N_CORES):
        o = np.asarray(results[c]["o"], dtype=np.float32).reshape(NSLOT, D + 1, QT)
        normed = o[:, :D, :] / o[:, D : D + 1, :]          # [NSLOT, 64, 512]
        normed = normed.transpose(0, 2, 1)                 # [NSLOT, 512, 64]
        for s, (b, qt) in enumerate(core_units[c]):
            out[b, qt * QT : (qt + 1) * QT, :] = normed[s]
    return out


def kernel(queries, keys, values, valid_lens):
    core_units, nv_slots = _plan(valid_lens)
    nc = _build_module(nv_slots)
    in_maps = _shard_inputs(queries, keys, values, valid_lens, core_units)
    res = bass_utils.run_bass_kernel_spmd(nc, in_maps, core_ids=list(range(N_CORES)))
    return _unpack(res.results, core_units)


# revision 19
# speedup vs baseline: 1.0527x; 1.0407x over previous
"""Fused masked-softmax attention (DotProductAttention) for 8 TRN2 NeuronCores.

Problem: B=16 batches of Q[2048,64] @ K[2048,64]^T -> mask cols >= valid_len
to -1e6 -> softmax -> @ V[2048,64].

Work decomposition: each batch splits into 4 q-quarters of 512 rows (one
PSUM-bank-wide q-tile each) -> 64 independent units.  Units are sorted by
valid k-tile count nv = ceil(valid_len/128) and dealt into 8 SPMD slots of
8 units (one per core); the compiled program runs slot s with a static
nv_s = max over that slot's units.  Masking and the k-tiles a unit runs
past its own valid_len are handled by HOST-side zeroing of the V-side
operand (see below), so skipped tiles are exact and extra tiles are 0.

Per-unit kernel (all on-chip, scores never touch HBM):
  * Layout: S^T[k, q] so softmax's k-reduction becomes a matmul and the
    attn @ V contraction needs no transpose of the big matrix.
  * mm1 PAIRS: the contraction is only D=64 deep, so TWO k-tiles run
    CONCURRENTLY in disjoint 64-row strips of the 128x128 PE array
    (tile_position row groups): kTa packs even k-tiles in partitions 0-63
    and odd k-tiles in 64-127, qTa holds Q^T duplicated in both halves.
    Each pair costs ~one matmul of wall time instead of two.
  * masking: exp(score) for a masked/padding position is multiplied by a
    HOST-zeroed Vaug row in mm2, contributing exactly 0 to numerator and
    denominator.  No mask row, no extra instructions.
  * exp:  ACT engine, exp(0.125 * x) straight out of PSUM in merged
    N<=1536 activations, bf16 out.
  * mm2:  O^T_aug [65, 512q] = sum_k Vaug[ktile].T @ expS^T[ktile] with
    Vaug = [V | ones] (bf16, masked rows zeroed) -> row 64 accumulates the
    softmax denominator in fp32 PSUM.
  * The schedule is software-pipelined one activation-group ahead across
    slot boundaries: PE emits mm1(group i+1) before mm2(group i), so the
    ACT engine never waits at a slot transition.
  * finish: one DVE copy PSUM->SBUF (bf16), one DMA of O^T_aug [65,512]
    per unit.  Normalization (divide by the denominator row) and the
    final transpose to [512,64] happen on the HOST after the gather.
"""

import functools

import numpy as np
import ml_dtypes

import concourse.bacc as bacc
import concourse.tile as tile
from concourse import mybir
from concourse import bass_utils

B, LQ, LKV, D = 16, 2048, 2048, 64
N_CORES = 8
KT = 128            # k-tile (partition dim of S^T)
QT = 512            # q-rows per unit (= PSUM bank free dim)
NKT = LKV // KT     # 16
NPMAX = NKT // 2    # max mm1 pairs per unit
NSLOT = (B * LQ) // (N_CORES * QT)  # 8 units per core
GROUP = 3           # max k-tiles per PSUM tile / merged activation
WARM_MM = 6         # bf16 warm-up matmuls to engage the HAM clock ramp
F32 = mybir.dt.float32
BF16 = mybir.dt.bfloat16


def _widths(nv, first_pair=False):
    """Split nv k-tiles into activation groups of width <=3, avoiding 1-wide
    groups (measured regression) where possible.  2-wide groups go FIRST so
    each slot's first activation has the shortest possible mm1 prefix.  The
    first slot processed leads with a 2-wide group: one mm1 PAIR produces
    both of its tiles, so the first exp fires after a single PE op."""
    if first_pair and nv > 2:
        return [2] + _widths(nv - 2)
    threes, rem = divmod(nv, 3)
    if rem == 0:
        return [3] * threes
    if rem == 2:
        return [2] + [3] * threes
    if threes >= 1:
        return [2, 2] + [3] * (threes - 1)
    return [nv]


@functools.lru_cache(maxsize=4)
def _build_module(nv_slots):
    nc = bacc.Bacc(None)
    qta_d = nc.dram_tensor("qta", [NSLOT, 128, QT], BF16, kind="ExternalInput")
    kta_d = nc.dram_tensor("kta", [NSLOT, 128, NPMAX * KT], BF16, kind="ExternalInput")
    vau_d = nc.dram_tensor("vaug", [128, NSLOT * NKT * (D + 1)], BF16, kind="ExternalInput")
    out_d = nc.dram_tensor("o", [NSLOT, D + 1, QT], BF16, kind="ExternalOutput")

    np_slots = [(nv + 1) // 2 for nv in nv_slots]

    slot_groups = []
    for si, nv in enumerate(nv_slots):
        groups, g = [], 0
        for w in _widths(nv, first_pair=(si == 0)):
            groups.append((g, w))
            g += w
        assert g == nv
        slot_groups.append(groups)

    with tile.TileContext(nc) as tc:
        with (
            tc.tile_pool(name="weights", bufs=1) as wpool,
            tc.tile_pool(name="exps", bufs=3) as epool,
            tc.tile_pool(name="outs", bufs=3) as opool,
            tc.tile_pool(name="ps_s", bufs=2, space="PSUM") as ps_s,
            tc.tile_pool(name="ps_o", bufs=2, space="PSUM") as ps_o,
        ):
            # Slots are sorted big -> small by _plan: start with the largest
            # (small prefetched head chunk), end with the smallest (minimal
            # exposed tail chain: last exp -> mm2s -> copy -> DMA).
            proc_order = list(range(NSLOT))

            kta_s = [
                wpool.tile([128, np_slots[s] * KT], BF16, tag=f"kta{s}", name=f"kta{s}")
                for s in range(NSLOT)
            ]
            qta_s = [
                wpool.tile([128, QT], BF16, tag=f"qta{s}", name=f"qta{s}")
                for s in range(NSLOT)
            ]
            vaug_s = [
                wpool.tile(
                    [128, nv_slots[s] * (D + 1)], BF16, tag=f"vaug{s}", name=f"vaug{s}"
                )
                for s in range(NSLOT)
            ]

            # PE warm-up: dense bf16 matmuls on a memset dummy tile while the
            # first input DMA is in flight (HAM clock ramps 1.2 -> 2.4 GHz
            # only after sustained PE activity).  GPSIMD finishes its preamble
            # first, so it does the memset; the count is capped so the last
            # warm matmul retires before the first input data can land.
            dummy = wpool.tile([128, QT], BF16, tag="dummy", name="dummy")
            nc.gpsimd.memset(dummy, 0)
            warm = ps_s.tile([128, GROUP * QT], F32, tag="st", name="warm")
            for _ in range(WARM_MM):
                nc.tensor.matmul(
                    warm[:, :QT], lhsT=dummy[:, :128], rhs=dummy, start=True, stop=True
                )

            # First slot's q tile and first kta pair-chunk go out in parallel
            # on the two HWDGE rings — together they gate the first matmul.
            nc.sync.dma_start(out=qta_s[0], in_=qta_d[0])
            nc.scalar.dma_start(out=kta_s[0][:, :KT], in_=kta_d[0, :, :KT])
            nc.sync.dma_start(
                out=kta_s[0][:, KT:], in_=kta_d[0, :, KT : np_slots[0] * KT]
            )
            nc.sync.dma_start(out=vaug_s[0], in_=vau_d[:, : nv_slots[0] * (D + 1)])
            for s in proc_order[1:]:
                nc.sync.dma_start(out=qta_s[s], in_=qta_d[s])
                nc.sync.dma_start(out=kta_s[s], in_=kta_d[s, :, : np_slots[s] * KT])
                nc.sync.dma_start(
                    out=vaug_s[s],
                    in_=vau_d[:, s * NKT * (D + 1) : (s * NKT + nv_slots[s]) * (D + 1)],
                )

            exps_t, po_t, st_t = {}, {}, {}
            emitted = []  # (s, g0, w) in exp emission order

            def mm2_group(s, g, w):
                nv = nv_slots[s]
                for j in range(w):
                    n = g + j
                    nc.tensor.matmul(
                        po_t[s],
                        lhsT=vaug_s[s][:, n * (D + 1) : (n + 1) * (D + 1)],
                        rhs=exps_t[s][:, n * QT : (n + 1) * QT],
                        start=(n == 0),
                        stop=(n == nv - 1),
                        skip_group_check=True,
                    )
                if g + w == nv:
                    # bf16 staging halves the output DMA; the host divides
                    # numerator by denominator in fp32 after the cast.
                    ob = opool.tile([D + 1, QT], BF16, tag="ob", name=f"ob{s}")
                    nc.vector.tensor_copy(ob, po_t[s])
                    nc.sync.dma_start(out=out_d[s], in_=ob)

            def emit_exp(s, gi):
                g, w = slot_groups[s][gi]
                nc.scalar.activation(
                    out=exps_t[s][:, g * QT : (g + w) * QT],
                    in_=st_t[(s, gi)][:, : w * QT],
                    func=mybir.ActivationFunctionType.Exp,
                    scale=0.125,
                )
                emitted.append((s, g, w))
                if len(emitted) >= 2:
                    mm2_group(*emitted[-2])

            for s in proc_order:
                nv = nv_slots[s]
                exps_t[s] = epool.tile([128, nv * QT], BF16, tag="exps", name=f"exps{s}")
                po_t[s] = ps_o.tile([D + 1, QT], F32, tag="po", name=f"po{s}")

                # tile index -> (group idx, slot-within-group)
                t2g = {}
                for gi, (g, w) in enumerate(slot_groups[s]):
                    for j in range(w):
                        t2g[g + j] = (gi, j)

                def st_slice(n):
                    gi, j = t2g[n]
                    if (s, gi) not in st_t:
                        st_t[(s, gi)] = ps_s.tile(
                            [128, GROUP * QT], F32, tag="st", name=f"st{s}_{gi}"
                        )
                    return st_t[(s, gi)][:, j * QT : (j + 1) * QT]

                # mm1 ops: pairs of k-tiles run concurrently in the two
                # 64-row strips of the PE array; odd nv leaves one single.
                for p in range((nv + 1) // 2):
                    n0, n1 = 2 * p, 2 * p + 1
                    outs = [st_slice(n0)] + ([st_slice(n1)] if n1 < nv else [])
                    nc.tensor.matmul(
                        outs[0],
                        lhsT=kta_s[s][0:64, p * KT : (p + 1) * KT],
                        rhs=qta_s[s][0:64, :],
                        start=True,
                        stop=True,
                    )
                    if n1 < nv:
                        nc.tensor.matmul(
                            outs[1],
                            lhsT=kta_s[s][64:128, p * KT : (p + 1) * KT],
                            rhs=qta_s[s][64:128, :],
                            start=True,
                            stop=True,
                        )
                    for n in (n0, n1):
                        if n < nv:
                            gi, j = t2g[n]
                            if j == slot_groups[s][gi][1] - 1:
                                emit_exp(s, gi)
            mm2_group(*emitted[-1])

    nc.compile()
    return nc


def _plan(valid_lens):
    """Sort the 64 (batch, q-quarter) units by valid k-tile count and deal
    them into NSLOT slots of one unit per core.  Returns (core_units,
    nv_slots) where core_units[c][s] = (batch, quarter)."""
    VL = np.asarray(valid_lens).astype(np.int64)
    nv = np.maximum(1, np.minimum(NKT, (VL + KT - 1) // KT))
    qpb = LQ // QT  # quarters per batch
    unit_nv = np.repeat(nv, qpb)
    order = np.argsort(-unit_nv, kind="stable")
    core_units = [
        [(int(order[NSLOT * s + c]) // qpb, int(order[NSLOT * s + c]) % qpb) for s in range(NSLOT)]
        for c in range(N_CORES)
    ]
    nv_slots = tuple(int(unit_nv[order[NSLOT * s]]) for s in range(NSLOT))
    return core_units, nv_slots


def _shard_inputs(queries, keys, values, valid_lens, core_units):
    """Host-side layout per core: stacked per-unit augmented operands."""
    Q = np.asarray(queries, dtype=np.float32)
    K = np.asarray(keys, dtype=np.float32)
    V = np.asarray(values, dtype=np.float32)
    VL = np.asarray(valid_lens).astype(np.int64)

    in_maps = []
    for c in range(N_CORES):
        qta = np.empty((NSLOT, 128, QT), np.float32)
        kta = np.zeros((NSLOT, 128, NPMAX * KT), np.float32)
        va = np.empty((128, NSLOT * NKT * (D + 1)), np.float32)
        for s, (b, qt) in enumerate(core_units[c]):
            qT = Q[b, qt * QT : (qt + 1) * QT, :].T          # [64, 512]
            qta[s, 0:64] = qT
            qta[s, 64:128] = qT
            tiles = K[b].T.reshape(D, NKT, KT)               # [64, 16, 128]
            kta[s, 0:64] = tiles[:, 0::2, :].reshape(D, -1)
            kta[s, 64:128] = tiles[:, 1::2, :].reshape(D, -1)
            # Vaug = [V | ones] with every row at or past valid_len zeroed:
            # masked/padding scores then contribute exactly 0 to both the
            # numerator and the denominator in mm2.
            vb = np.concatenate([V[b], np.ones((LKV, 1), np.float32)], axis=-1)
            vb[VL[b] :, :] = 0.0
            va[:, s * NKT * (D + 1) : (s + 1) * NKT * (D + 1)] = (
                vb.reshape(NKT, KT, D + 1).transpose(1, 0, 2).reshape(128, -1)
            )
        in_maps.append(
            {
                "qta": qta.astype(ml_dtypes.bfloat16),
                "kta": kta.astype(ml_dtypes.bfloat16),
                "vaug": va.astype(ml_dtypes.bfloat16),
            }
        )
    return in_maps


def _unpack(results, core_units):
    """Host-side finish: normalize by the accumulated denominator row and
    transpose each unit's O^T_aug [65, 512] back to [512, 64]."""
    out = np.empty((B, LQ, D), np.float32)
    for c in range(N_CORES):
        o = np.asarray(results[c]["o"], dtype=np.float32).reshape(NSLOT, D + 1, QT)
        normed = o[:, :D, :] / o[:, D : D + 1, :]          # [NSLOT, 64, 512]
        normed = normed.transpose(0, 2, 1)                 # [NSLOT, 512, 64]
        for s, (b, qt) in enumerate(core_units[c]):
            out[b, qt * QT : (qt + 1) * QT, :] = normed[s]
    return out


def kernel(queries, keys, values, valid_lens):
    core_units, nv_slots = _plan(valid_lens)
    nc = _build_module(nv_slots)
    in_maps = _shard_inputs(queries, keys, values, valid_lens, core_units)
    res = bass_utils.run_bass_kernel_spmd(nc, in_maps, core_ids=list(range(N_CORES)))
    return _unpack(res.results, core_units)


# revision 23
# speedup vs baseline: 1.0577x; 1.0048x over previous
"""Fused masked-softmax attention (DotProductAttention) for 8 TRN2 NeuronCores.

Problem: B=16 batches of Q[2048,64] @ K[2048,64]^T -> mask cols >= valid_len
to -1e6 -> softmax -> @ V[2048,64].

Work decomposition: each batch splits into 4 q-quarters of 512 rows (one
PSUM-bank-wide q-tile each) -> 64 independent units.  Units are sorted by
valid k-tile count nv = ceil(valid_len/128) and dealt into 8 SPMD slots of
8 units (one per core); the compiled program runs slot s with a static
nv_s = max over that slot's units.  Masking and the k-tiles a unit runs
past its own valid_len are handled by HOST-side zeroing of the V-side
operand (see below), so skipped tiles are exact and extra tiles are 0.

Per-unit kernel (all on-chip, scores never touch HBM):
  * Layout: S^T[k, q] so softmax's k-reduction becomes a matmul and the
    attn @ V contraction needs no transpose of the big matrix.
  * mm1 PAIRS: the contraction is only D=64 deep, so TWO k-tiles run
    CONCURRENTLY in disjoint 64-row strips of the 128x128 PE array
    (tile_position row groups): kTa packs even k-tiles in partitions 0-63
    and odd k-tiles in 64-127, qTa holds Q^T duplicated in both halves.
    Each pair costs ~one matmul of wall time instead of two.
  * masking: exp(score) for a masked/padding position is multiplied by a
    HOST-zeroed Vaug row in mm2, contributing exactly 0 to numerator and
    denominator.  No mask row, no extra instructions.
  * exp:  ACT engine, exp(0.125 * x) straight out of PSUM in merged
    N<=1536 activations, bf16 out.
  * mm2:  O^T_aug [65, 512q] = sum_k Vaug[ktile].T @ expS^T[ktile] with
    Vaug = [V | ones] (bf16, masked rows zeroed) -> row 64 accumulates the
    softmax denominator in fp32 PSUM.
  * The schedule is software-pipelined one activation-group ahead across
    slot boundaries: PE emits mm1(group i+1) before mm2(group i), so the
    ACT engine never waits at a slot transition.
  * finish: one DVE copy PSUM->SBUF (bf16), one DMA of O^T_aug [65,512]
    per unit.  Normalization (divide by the denominator row) and the
    final transpose to [512,64] happen on the HOST after the gather.
"""

import functools

import numpy as np
import ml_dtypes

import concourse.bacc as bacc
import concourse.tile as tile
from concourse import mybir
from concourse import bass_utils

B, LQ, LKV, D = 16, 2048, 2048, 64
N_CORES = 8
KT = 128            # k-tile (partition dim of S^T)
QT = 512            # q-rows per unit (= PSUM bank free dim)
NKT = LKV // KT     # 16
NPMAX = NKT // 2    # max mm1 pairs per unit
NSLOT = (B * LQ) // (N_CORES * QT)  # 8 units per core
GROUP = 3           # max k-tiles per PSUM tile / merged activation
WARM_MM = 6         # bf16 warm-up matmuls to engage the HAM clock ramp
F32 = mybir.dt.float32
BF16 = mybir.dt.bfloat16


def _widths(nv, first_pair=False):
    """Split nv k-tiles into activation groups of width <=3, avoiding 1-wide
    groups (measured regression) where possible.  2-wide groups go FIRST so
    each slot's first activation has the shortest possible mm1 prefix.  The
    first slot processed leads with a 2-wide group: one mm1 PAIR produces
    both of its tiles, so the first exp fires after a single PE op."""
    if first_pair and nv > 2:
        return [2] + _widths(nv - 2)
    threes, rem = divmod(nv, 3)
    if rem == 0:
        return [3] * threes
    if rem == 2:
        return [2] + [3] * threes
    if threes >= 1:
        return [2, 2] + [3] * (threes - 1)
    return [nv]


@functools.lru_cache(maxsize=4)
def _build_module(nv_slots):
    nc = bacc.Bacc(None)
    qta_d = nc.dram_tensor("qta", [NSLOT, 128, QT], BF16, kind="ExternalInput")
    kta_d = nc.dram_tensor("kta", [NSLOT, 128, NPMAX * KT], BF16, kind="ExternalInput")
    vau_d = nc.dram_tensor("vaug", [128, NSLOT * NKT * (D + 1)], BF16, kind="ExternalInput")
    out_d = nc.dram_tensor("o", [NSLOT, D + 1, QT], BF16, kind="ExternalOutput")

    np_slots = [(nv + 1) // 2 for nv in nv_slots]

    slot_groups = []
    for si, nv in enumerate(nv_slots):
        groups, g = [], 0
        for w in _widths(nv, first_pair=(si == 0)):
            groups.append((g, w))
            g += w
        assert g == nv
        slot_groups.append(groups)

    with tile.TileContext(nc) as tc:
        with (
            tc.tile_pool(name="weights", bufs=1) as wpool,
            tc.tile_pool(name="exps", bufs=1) as epool,
            tc.tile_pool(name="outs", bufs=3) as opool,
            tc.tile_pool(name="ps_s", bufs=2, space="PSUM") as ps_s,
            tc.tile_pool(name="ps_o", bufs=2, space="PSUM") as ps_o,
        ):
            # Slots are sorted big -> small by _plan: start with the largest
            # (small prefetched head chunk), end with the smallest (minimal
            # exposed tail chain: last exp -> mm2s -> copy -> DMA).
            proc_order = list(range(NSLOT))

            kta_s = [
                wpool.tile([128, np_slots[s] * KT], BF16, tag=f"kta{s}", name=f"kta{s}")
                for s in range(NSLOT)
            ]
            qta_s = [
                wpool.tile([128, QT], BF16, tag=f"qta{s}", name=f"qta{s}")
                for s in range(NSLOT)
            ]
            vaug_s = [
                wpool.tile(
                    [128, nv_slots[s] * (D + 1)], BF16, tag=f"vaug{s}", name=f"vaug{s}"
                )
                for s in range(NSLOT)
            ]

            # PE warm-up: dense bf16 matmuls on a memset dummy tile while the
            # first input DMA is in flight (HAM clock ramps 1.2 -> 2.4 GHz
            # only after sustained PE activity).  GPSIMD finishes its preamble
            # first, so it does the memset; the count is capped so the last
            # warm matmul retires before the first input data can land.
            dummy = wpool.tile([128, QT], BF16, tag="dummy", name="dummy")
            nc.gpsimd.memset(dummy, 0)
            warm = ps_s.tile([128, GROUP * QT], F32, tag="st", name="warm")
            for _ in range(WARM_MM):
                nc.tensor.matmul(
                    warm[:, :QT], lhsT=dummy[:, :128], rhs=dummy, start=True, stop=True
                )

            # First slot's q tile and first kta pair-chunk go out in parallel
            # on the two HWDGE rings — together they gate the first matmul.
            nc.sync.dma_start(out=qta_s[0], in_=qta_d[0])
            nc.scalar.dma_start(out=kta_s[0][:, :KT], in_=kta_d[0, :, :KT])
            nc.sync.dma_start(
                out=kta_s[0][:, KT:], in_=kta_d[0, :, KT : np_slots[0] * KT]
            )
            nc.sync.dma_start(out=vaug_s[0], in_=vau_d[:, : nv_slots[0] * (D + 1)])
            for s in proc_order[1:]:
                nc.sync.dma_start(out=qta_s[s], in_=qta_d[s])
                nc.sync.dma_start(out=kta_s[s], in_=kta_d[s, :, : np_slots[s] * KT])
                nc.sync.dma_start(
                    out=vaug_s[s],
                    in_=vau_d[:, s * NKT * (D + 1) : (s * NKT + nv_slots[s]) * (D + 1)],
                )

            exps_t, po_t, st_t = {}, {}, {}

            def mm2_group(s, g, w):
                nv = nv_slots[s]
                for j in range(w):
                    n = g + j
                    nc.tensor.matmul(
                        po_t[s],
                        lhsT=vaug_s[s][:, n * (D + 1) : (n + 1) * (D + 1)],
                        rhs=exps_t[s][:, n * QT : (n + 1) * QT],
                        start=(n == 0),
                        stop=(n == nv - 1),
                        skip_group_check=True,
                    )
                if g + w == nv:
                    # bf16 staging halves the output DMA; the host divides
                    # numerator by denominator in fp32 after the cast.
                    ob = opool.tile([D + 1, QT], BF16, tag="ob", name=f"ob{s}")
                    nc.vector.tensor_copy(ob, po_t[s])
                    nc.sync.dma_start(out=out_d[s], in_=ob)

            def emit_exp(s, gi):
                g, w = slot_groups[s][gi]
                nc.scalar.activation(
                    out=exps_t[s][:, g * QT : (g + w) * QT],
                    in_=st_t[(s, gi)][:, : w * QT],
                    func=mybir.ActivationFunctionType.Exp,
                    scale=0.125,
                )

            # Phase 1: the full mm1 + exp stream.  ALL mm2 work is deferred:
            # while the HAM clock gate still has the PE at 1.2 GHz, the PE
            # only has to feed the ACT engine with cheap paired mm1s (~640ns
            # per 3-tile group vs ACT's ~1580ns), so exp never starves even
            # cold.  Every slot's exp tile stays live in SBUF (per-slot tag).
            for s in proc_order:
                nv = nv_slots[s]
                exps_t[s] = epool.tile(
                    [128, nv * QT], BF16, tag=f"exps{s}", name=f"exps{s}"
                )

                # tile index -> (group idx, slot-within-group)
                t2g = {}
                for gi, (g, w) in enumerate(slot_groups[s]):
                    for j in range(w):
                        t2g[g + j] = (gi, j)

                def st_slice(n):
                    gi, j = t2g[n]
                    if (s, gi) not in st_t:
                        st_t[(s, gi)] = ps_s.tile(
                            [128, GROUP * QT], F32, tag="st", name=f"st{s}_{gi}"
                        )
                    return st_t[(s, gi)][:, j * QT : (j + 1) * QT]

                # mm1 ops: pairs of k-tiles run concurrently in the two
                # 64-row strips of the PE array; odd nv leaves one single.
                for p in range((nv + 1) // 2):
                    n0, n1 = 2 * p, 2 * p + 1
                    outs = [st_slice(n0)] + ([st_slice(n1)] if n1 < nv else [])
                    nc.tensor.matmul(
                        outs[0],
                        lhsT=kta_s[s][0:64, p * KT : (p + 1) * KT],
                        rhs=qta_s[s][0:64, :],
                        start=True,
                        stop=True,
                    )
                    if n1 < nv:
                        nc.tensor.matmul(
                            outs[1],
                            lhsT=kta_s[s][64:128, p * KT : (p + 1) * KT],
                            rhs=qta_s[s][64:128, :],
                            start=True,
                            stop=True,
                        )
                    for n in (n0, n1):
                        if n < nv:
                            gi, j = t2g[n]
                            if j == slot_groups[s][gi][1] - 1:
                                emit_exp(s, gi)

            # Phase 2: the deferred mm2 accumulations + finishes.  These
            # execute in the PE's slack behind the exp stream (warm-phase PE
            # is ~3x faster than ACT per k-tile), so the kernel end stays
            # pinned to the last exp + one group of mm2 + copy + DMA.
            for s in proc_order:
                po_t[s] = ps_o.tile([D + 1, QT], F32, tag="po", name=f"po{s}")
                for g, w in slot_groups[s]:
                    mm2_group(s, g, w)

    nc.compile()
    return nc


def _plan(valid_lens):
    """Sort the 64 (batch, q-quarter) units by valid k-tile count and deal
    them into NSLOT slots of one unit per core.  Returns (core_units,
    nv_slots) where core_units[c][s] = (batch, quarter)."""
    VL = np.asarray(valid_lens).astype(np.int64)
    nv = np.maximum(1, np.minimum(NKT, (VL + KT - 1) // KT))
    qpb = LQ // QT  # quarters per batch
    unit_nv = np.repeat(nv, qpb)
    order = np.argsort(-unit_nv, kind="stable")
    core_units = [
        [(int(order[NSLOT * s + c]) // qpb, int(order[NSLOT * s + c]) % qpb) for s in range(NSLOT)]
        for c in range(N_CORES)
    ]
    nv_slots = tuple(int(unit_nv[order[NSLOT * s]]) for s in range(NSLOT))
    return core_units, nv_slots


def _shard_inputs(queries, keys, values, valid_lens, core_units):
    """Host-side layout per core: stacked per-unit augmented operands."""
    Q = np.asarray(queries, dtype=np.float32)
    K = np.asarray(keys, dtype=np.float32)
    V = np.asarray(values, dtype=np.float32)
    VL = np.asarray(valid_lens).astype(np.int64)

    in_maps = []
    for c in range(N_CORES):
        qta = np.empty((NSLOT, 128, QT), np.float32)
        kta = np.zeros((NSLOT, 128, NPMAX * KT), np.float32)
        va = np.empty((128, NSLOT * NKT * (D + 1)), np.float32)
        for s, (b, qt) in enumerate(core_units[c]):
            qT = Q[b, qt * QT : (qt + 1) * QT, :].T          # [64, 512]
            qta[s, 0:64] = qT
            qta[s, 64:128] = qT
            tiles = K[b].T.reshape(D, NKT, KT)               # [64, 16, 128]
            kta[s, 0:64] = tiles[:, 0::2, :].reshape(D, -1)
            kta[s, 64:128] = tiles[:, 1::2, :].reshape(D, -1)
            # Vaug = [V | ones] with every row at or past valid_len zeroed:
            # masked/padding scores then contribute exactly 0 to both the
            # numerator and the denominator in mm2.
            vb = np.concatenate([V[b], np.ones((LKV, 1), np.float32)], axis=-1)
            vb[VL[b] :, :] = 0.0
            va[:, s * NKT * (D + 1) : (s + 1) * NKT * (D + 1)] = (
                vb.reshape(NKT, KT, D + 1).transpose(1, 0, 2).reshape(128, -1)
            )
        in_maps.append(
            {
                "qta": qta.astype(ml_dtypes.bfloat16),
                "kta": kta.astype(ml_dtypes.bfloat16),
                "vaug": va.astype(ml_dtypes.bfloat16),
            }
        )
    return in_maps


def _unpack(results, core_units):
    """Host-side finish: normalize by the accumulated denominator row and
    transpose each unit's O^T_aug [65, 512] back to [512, 64]."""
    out = np.empty((B, LQ, D), np.float32)
    for c in range(N_CORES):
        o = np.asarray(results[c]["o"], dtype=np.float32).reshape(NSLOT, D + 1, QT)
        normed = o[:, :D, :] / o[:, D : D + 1, :]          # [NSLOT, 64, 512]
        normed = normed.transpose(0, 2, 1)                 # [NSLOT, 512, 64]
        for s, (b, qt) in enumerate(core_units[c]):
            out[b, qt * QT : (qt + 1) * QT, :] = normed[s]
    return out


def kernel(queries, keys, values, valid_lens):
    core_units, nv_slots = _plan(valid_lens)
    nc = _build_module(nv_slots)
    in_maps = _shard_inputs(queries, keys, values, valid_lens, core_units)
    res = bass_utils.run_bass_kernel_spmd(nc, in_maps, core_ids=list(range(N_CORES)))
    return _unpack(res.results, core_units)
